# revision 1
# baseline (speedup 1.0000x reference)
"""Trainium2 Bass kernel for CDRExtractor (segment_reduce).

Input : segmentation_mask (64, 3, 512, 512) fp32
Output: (64, 5) fp32 = [cdr, disc_mean, cup_mean, disc_mean, cup_mean]

Sharding: pure data parallel, 8 samples per core across 8 cores.

v2 design, 76.81us (vs 95.0us single-queue baseline).  Key insight: the
CoreSim cost model prices each DMA on the ISSUING engine queue (SP /
Activation / Pool run their transfers concurrently), so the 75.8us of
per-queue DMA work for the 24 MiB shard is split  SP ~54 / Pool ~16 /
ACT ~5 us  and overlaps compute instead of gating it.  Per (sample,
channel) plane loads land as (128, 4, 512) tiles: partition = h%128,
free = (e=h//128, w) via a `(e p) w -> p e w` source AP.

Per-sample math (bf16 after the first subtract):
  T = [x1-x0 | x2-x0]       Pool TT sub, fp32 in -> bf16 out (2 instrs)
  F = exp(T)                ACT, two (128,2048) instrs (queue granularity)
  sadd = f1+f2              DVE TT (2x bf16 mode)
  r = 1/(1+sadd):           L = ln(1+sadd); r = exp(-L) on ACT, with the
                            row-sum of r fused into the exp (accum_out)
  p1 row-sums:              pscr = f1*r (DVE TT 2x) + tensor_scalar
                            mult/add accum at 4x -> RS1 col per sample
  p2 row-sums:              via identity sum(p0+p1+p2) = N in the tail
  d1 (cup presence):        one fused STT per (s, e-block):
                            count[max(t2,0) < t1], exact argmax test
  d2 (disc presence):       m2 = max(t1,0) (DVE tensor_scalar 4x),
                            g2 = t2-m2 (Pool), count[g2>0] via
                            tensor_scalar is_gt/add accum (DVE 4x)
  tail: ones-matmul over (128,32) accumulators (col = s*4+e),
        iota+penalty reduce-min/max for ymin/ymax per (sample, label),
        heights = relu(ymax-ymin), cdr = h_cup/(h_disc+1e-6), means /HW.

Scheduling (engine queues are in-order; emission order = queue order):
  - software pipeline over samples: front(i-2) / mid(i-3) / back(i-4),
    loads emitted last each iteration; d-counts ride in the mid stage
    (they only need T, giving DVE ready work between sadd and p1).
  - s0 loads+subs+exp run in 128-row chunks across all three DMA queues
    to shorten fill; s1's planes are spread across 3 queues.
  - s7 (drain sample) runs fully chunked, its chunk loads interleaved
    into the tail of the SP queue (iterations 4..7) and its
    front/mid/back woven per-chunk into iterations 6..11, so its chain
    overlaps the other engines' backlog.
  - engine busy (CoreSim): ACT 71.5 / DVE 55.4 / Pool 53.9 / SP 62.8 us;
    ACT-heavy on purpose - configs that balanced busy by moving
    denominator work to DVE (reciprocal path) measured WORSE end-to-end
    because DVE sits on the dependency-critical path and needs slack.

Numerics: bf16 internals; rel err vs fp32 reference ~2e-4 (gate 2e-2).
The d tests are exact in bf16-rounded t-space; row presence is
overwhelming for N(0,1) inputs so heights/cdr are robust.
HW-verified rel err via test.py: 2.03e-04, HW exec 76811 ns.
"""

import numpy as np
from contextlib import ExitStack

B, C, H, W = 64, 3, 512, 512
NCORES = 8
SPC = B // NCORES      # samples per core = 8
NB = H // 128          # 128-row blocks = 4
HW = float(H * W)

_CACHE = {}

# ---- knobs ----
CFG = dict(
    chunk=("s0", "s7"),            # which of s0/s7 run chunked
    s0_front_only=True,            # s0: chunk only loads+subs+exp
    denom={},                      # per-sample: 'act'|'dve'|'div'(sim-only)
    d1={},                         # per-sample 'fused' (default) | 'B'
    d2={s: "B" for s in range(8)},  # per-sample 'fused' | 'B'
    sadd_eng={6: "pool", 7: "pool"},  # per-sample 'dve'|'pool'
    pool_planes=(3,),              # samples whose c0 plane loads on Pool
    act_chunks=3,                  # how many s0c2 chunk loads on ACT queue
    consts_q="sp_late",            # 'act' | 'pool' | 'sp_late'
    s7_interleave=True,            # spread s7 chunk loads through SP queue
    tail_tt_eng="dve",             # cmin/cmax engine
    load_pos="last",               # 'split' | 'last'
    d_stage="mid",                 # 'back' | 'mid'
    spread_early=(1,),             # samples loaded across 3 queues
    stage_order="fmb",             # 'fmb' | 'mfb' | 'mbf'
    weave_s7=True,                 # interleave s7 chunk stages
    weave_lag=6,                   # iteration of s7 chunk-0 front
    s7_il_start=4,                 # iteration of first s7 chunk load
    sr_eng="act",                  # sum(r) via 'act' accum | 'dve' ts
    denom_split=0,                 # e-blocks per sample on DVE recip
    s7_mid_half=False,             # s7 denominator at half granularity
    s0_exp_whole=False,            # s0: single full-tile exp
    dma_sub=False,                 # fuse t-subs into accum DMAs
    bufs=(3, 4, 3, 2, 3),          # xpool,tpool,fpool,mpool,rpool
    weave_s6=False,                # also chunk+weave sample 6
    s6_lag=5,                      # iteration of s6 chunk-0 front
    s6_il_start=3,                 # iteration of first s6 chunk load
    drain_d_first=False,           # emit d before p in woven back
    s7_dsplit=0,                   # last k e-blocks of s7 denom on DVE
    denom_mode="lnexp",            # 'lnexp' (ACT); 'pow' fails HW ISA
    act_planes=(),                 # (s,c) planes loaded on ACT queue
    exp_split=True,                # exp as two (128,2048) instrs
    lnr_split=False,               # ln/r as two (128,1024) instrs
    back_lag=4,                    # iteration lag of the back stage
    tail_split=False,              # early mean-chain for samples 0-5
    pool_chan=0,                   # which channel of pool_planes samples
)


def _build():
    import concourse.bass as bass
    import concourse.bacc as bacc
    import concourse.mybir as mybir
    from concourse.tile import TileContext

    if not _CACHE.get("act_patch"):
        _orig_tables = bacc.get_activation_tables

        def _only_ln_exp(arch):
            t = _orig_tables(arch)
            keep = "natural_log_exp_and_others"
            return {k: (v if k == keep else set()) for k, v in t.items()}

        bacc.get_activation_tables = _only_ln_exp
        _CACHE["act_patch"] = True

    f32 = mybir.dt.float32
    bf16 = mybir.dt.bfloat16
    Alu = mybir.AluOpType
    AFT = mybir.ActivationFunctionType
    X_AX = mybir.AxisListType.X

    CHUNKED = set()
    if "s0" in CFG["chunk"]:
        CHUNKED.add(0)
    if "s7" in CFG["chunk"]:
        CHUNKED.add(SPC - 1)
    if CFG["weave_s6"]:
        CHUNKED.add(SPC - 2)

    nc = bacc.Bacc()
    x = nc.dram_tensor("x", (SPC, C, H, W), f32, kind="ExternalInput")
    iota_in = nc.dram_tensor("iota", (32, 128), f32, kind="ExternalInput")
    ident_in = nc.dram_tensor("ident", (128, 128), f32, kind="ExternalInput")
    ones_in = nc.dram_tensor("ones", (128, 1), f32, kind="ExternalInput")
    out = nc.dram_tensor("out", (5, SPC), f32, kind="ExternalOutput")

    with TileContext(nc) as tc, ExitStack() as ctx:
        QENG = dict(sp=nc.sync, act=nc.scalar, pool=nc.gpsimd)
        cpool = ctx.enter_context(tc.tile_pool(name="consts", bufs=1))
        apool = ctx.enter_context(tc.tile_pool(name="accs", bufs=1))
        bx, bt, bf, bm, br = CFG["bufs"]
        xpool = ctx.enter_context(tc.tile_pool(name="xin", bufs=bx))
        tpool = ctx.enter_context(tc.tile_pool(name="tmain", bufs=bt))
        fpool = ctx.enter_context(tc.tile_pool(name="fmain", bufs=bf))
        mpool = ctx.enter_context(tc.tile_pool(name="mid", bufs=bm))
        rpool = ctx.enter_context(tc.tile_pool(name="rr", bufs=br))
        ppool = ctx.enter_context(tc.tile_pool(name="ps", bufs=1,
                                               space="PSUM"))

        warm = cpool.tile([1, 16], bf16, tag="warm")
        nc.vector.memset(warm[:, :], 0.0)
        nc.scalar.activation(warm[:, :], warm[:, :], AFT.Exp)

        iota = cpool.tile([32, 128], f32, tag="iota")
        ident = cpool.tile([128, 128], f32, tag="ident")
        ones = cpool.tile([128, 1], f32, tag="ones")

        def emit_consts():
            q = CFG["consts_q"]
            q = {"sp_late": "sp", "pool_late": "pool"}.get(q, q)
            cq = QENG[q]
            cq.dma_start(iota[:, :], iota_in[:, :])
            cq.dma_start(ident[:, :], ident_in[:, :])
            cq.dma_start(ones[:, :], ones_in[:, :])

        if CFG["consts_q"] not in ("sp_late", "pool_late"):
            emit_consts()

        # accumulators: col j = s*4 + e  (e = h//128 block)
        RS1 = apool.tile([128, 32], f32, tag="RS1")
        RSr = apool.tile([128, 32], f32, tag="RSr")
        DM1 = apool.tile([128, 32], f32, tag="DM1")
        DM2 = apool.tile([128, 32], f32, tag="DM2")
        for acc_t in (RS1, RSr, DM1, DM2):
            nc.vector.memset(acc_t[:, :], 0.0)

        junkA = cpool.tile([128, 2048], bf16, tag="junkA")
        junkB = cpool.tile([128, 2048], bf16, tag="junkB")

        X = {}
        Tt = {}
        Ff = {}
        Rr = {}

        def eslice(base, e):
            return slice(base + e * 512, base + (e + 1) * 512)

        def load_plane(s, c, qname="sp"):
            if (s, c) in CFG["act_planes"]:
                qname = "act"
            Xt = xpool.tile([128, NB, 512], f32, tag=f"X{c}",
                            name=f"X_{s}_{c}")
            X[(s, c)] = Xt
            src = x[s, c].rearrange("(e p) w -> p e w", p=128)
            QENG[qname].dma_start(Xt, src)

        def load_chunk(s, c, e, q):
            if (s, c) not in X:
                X[(s, c)] = xpool.tile([128, NB, 512], f32, tag=f"X{c}",
                                       name=f"X_{s}_{c}")
            src = x[s, c, e * 128:(e + 1) * 128, :]
            QENG[q].dma_start(X[(s, c)][:, e, :], src)

        tdt = f32 if CFG["dma_sub"] else bf16

        def _tdst(s, half):
            if s not in Tt:
                Tt[s] = tpool.tile([128, 4096], tdt, tag="T", name=f"T_{s}")
                Ff[s] = fpool.tile([128, 4096], bf16, tag="F", name=f"F_{s}")
            dst = Tt[s][:, half * 2048:(half + 1) * 2048]
            return dst.rearrange("p (e w) -> p e w", e=NB)

        def load_sub_base(s, half):
            nc.sync.dma_start(
                _tdst(s, half), x[s, 0].rearrange("(e p) w -> p e w", p=128))

        def load_sub_accum(s, half):
            nc.gpsimd.dma_start(
                _tdst(s, half),
                x[s, 1 + half].rearrange("(e p) w -> p e w", p=128),
                accum_op=Alu.subtract)

        def stage_front(s, erange=None):
            """T = [x1-x0 | x2-x0] (POOL), F = exp(T) (ACT)."""
            if s not in Tt:
                Tt[s] = tpool.tile([128, 4096], tdt, tag="T", name=f"T_{s}")
                Ff[s] = fpool.tile([128, 4096], bf16, tag="F", name=f"F_{s}")
            T, F = Tt[s], Ff[s]
            if CFG["dma_sub"]:
                nc.scalar.activation(F[:, :], T[:, :], AFT.Exp)
                return
            x0 = X[(s, 0)].rearrange("p e w -> p (e w)")
            x1 = X[(s, 1)].rearrange("p e w -> p (e w)")
            x2 = X[(s, 2)].rearrange("p e w -> p (e w)")
            if erange is None:
                nc.gpsimd.tensor_tensor(T[:, 0:2048], x1, x0, Alu.subtract)
                nc.gpsimd.tensor_tensor(T[:, 2048:4096], x2, x0,
                                        Alu.subtract)
                if CFG["exp_split"]:
                    nc.scalar.activation(F[:, 0:2048], T[:, 0:2048], AFT.Exp)
                    nc.scalar.activation(F[:, 2048:4096], T[:, 2048:4096],
                                         AFT.Exp)
                else:
                    nc.scalar.activation(F[:, :], T[:, :], AFT.Exp)
                return
            Tv = T.rearrange("p (l e w) -> p l e w", l=2, e=NB)
            Fv = F.rearrange("p (l e w) -> p l e w", l=2, e=NB)
            whole_exp = s == 0 and CFG["s0_exp_whole"]
            for e in erange:
                nc.gpsimd.tensor_tensor(
                    T[:, eslice(0, e)], x1[:, eslice(0, e)],
                    x0[:, eslice(0, e)], Alu.subtract)
                nc.gpsimd.tensor_tensor(
                    T[:, eslice(2048, e)], x2[:, eslice(0, e)],
                    x0[:, eslice(0, e)], Alu.subtract)
                if not whole_exp:
                    nc.scalar.activation(Fv[:, :, e, :], Tv[:, :, e, :],
                                         AFT.Exp)
            if whole_exp and (NB - 1) in erange:
                nc.scalar.activation(F[:, :], T[:, :], AFT.Exp)

        def stage_mid(s, erange=None):
            """sadd = f1+f2, denominator r (+ row-sums of r)."""
            T, F = Tt[s], Ff[s]
            if s not in Rr:
                Rr[s] = rpool.tile([128, 2048], bf16, tag="r", name=f"r_{s}")
                Rr[(s, "sadd")] = mpool.tile([128, 2048], bf16, tag="sadd",
                                             name=f"sa_{s}")
                if CFG["denom"].get(s, "act") == "dve":
                    Rr[(s, "aux")] = mpool.tile([128, 2048], bf16, tag="s1p",
                                                name=f"s1p_{s}")
                else:
                    Rr[(s, "aux")] = mpool.tile([128, 2048], bf16, tag="L",
                                                name=f"L_{s}")
            r, sadd, aux = Rr[s], Rr[(s, "sadd")], Rr[(s, "aux")]
            se = CFG["sadd_eng"]
            se = se.get(s, "dve") if isinstance(se, dict) else se
            saddf = nc.vector if se == "dve" else nc.gpsimd
            nsp = CFG["denom_split"]
            half = (s == s_last and CFG["s7_mid_half"]
                    and erange is not None)
            if half:
                erange = [e for e in erange if e % 2 == 0]
            for e in ([None] if erange is None else erange):
                if e is None:
                    sl = slice(0, 2048)
                elif half:
                    sl = slice(e * 512, (e + 2) * 512)
                else:
                    sl = eslice(0, e)
                col = s * 4 + (0 if e is None else e)
                saddf.tensor_tensor(
                    sadd[:, sl], F[:, sl],
                    F[:, 2048 + sl.start:2048 + sl.stop], Alu.add)
                if (e is None and nsp > 0
                        and CFG["denom"].get(s, "act") == "act"):
                    # split: ACT ln/r on blocks [0, 4-nsp), DVE recip on rest
                    cut = (NB - nsp) * 512
                    a_sl = slice(0, cut)
                    d_sl = slice(cut, 2048)
                    nc.scalar.activation(aux[:, a_sl], sadd[:, a_sl],
                                         AFT.Ln, bias=1.0)
                    nc.scalar.activation(r[:, a_sl], aux[:, a_sl], AFT.Exp,
                                         scale=-1.0,
                                         accum_out=RSr[:, col:col + 1])
                    nc.vector.tensor_scalar_add(aux[:, d_sl], sadd[:, d_sl],
                                                1.0)
                    with nc.allow_low_precision(reason="bf16 denom"):
                        nc.vector.reciprocal(r[:, d_sl], aux[:, d_sl])
                    nc.vector.tensor_scalar(
                        junkA[:, d_sl], r[:, d_sl], 1.0, 0.0, Alu.mult,
                        Alu.add, accum_out=RSr[:, col + 3:col + 4])
                    continue
                dve_here = CFG["denom"].get(s, "act") == "dve"
                if (s == s_last and e is not None
                        and e >= NB - CFG["s7_dsplit"]):
                    dve_here = True
                if (CFG["denom_mode"] == "div"
                        or CFG["denom"].get(s) == "div"):
                    # only s1p = 1+sadd needed; p1/p2 divide by it in back
                    nc.vector.tensor_scalar_add(r[:, sl], sadd[:, sl], 1.0)
                    continue
                if CFG["denom_mode"] == "pow":
                    # r = (1+sadd)^-1 on DVE at 4x; sum(r) via a separate
                    # mult/add accumulate (pow+accum fails the neuronxcc
                    # tensor_scalar_cache_reduce_valid_ops ISA check)
                    nc.vector.tensor_scalar_add(aux[:, sl], sadd[:, sl], 1.0)
                    nc.vector.tensor_scalar(
                        r[:, sl], aux[:, sl], -1.0, None, Alu.pow)
                    nc.vector.tensor_scalar(
                        junkA[:, sl], r[:, sl], 1.0, 0.0, Alu.mult, Alu.add,
                        accum_out=RSr[:, col:col + 1])
                elif dve_here:
                    nc.vector.tensor_scalar_add(aux[:, sl], sadd[:, sl], 1.0)
                    with nc.allow_low_precision(reason="bf16 softmax denom"):
                        nc.vector.reciprocal(r[:, sl], aux[:, sl])
                    nc.vector.tensor_scalar(
                        junkA[:, sl], r[:, sl], 1.0, 0.0, Alu.mult, Alu.add,
                        accum_out=RSr[:, col:col + 1])
                else:
                    if (CFG["lnr_split"] and e is None
                            and CFG["sr_eng"] == "act"):
                        for hh in (0, 1):
                            hsl = slice(hh * 1024, (hh + 1) * 1024)
                            hcol = col + 2 * hh
                            nc.scalar.activation(aux[:, hsl], sadd[:, hsl],
                                                 AFT.Ln, bias=1.0)
                            nc.scalar.activation(
                                r[:, hsl], aux[:, hsl], AFT.Exp, scale=-1.0,
                                accum_out=RSr[:, hcol:hcol + 1])
                        continue
                    nc.scalar.activation(aux[:, sl], sadd[:, sl], AFT.Ln,
                                         bias=1.0)
                    if CFG["sr_eng"] == "act":
                        nc.scalar.activation(r[:, sl], aux[:, sl], AFT.Exp,
                                             scale=-1.0,
                                             accum_out=RSr[:, col:col + 1])
                    else:
                        nc.scalar.activation(r[:, sl], aux[:, sl], AFT.Exp,
                                             scale=-1.0)
                        nc.vector.tensor_scalar(
                            junkA[:, sl], r[:, sl], 1.0, 0.0, Alu.mult,
                            Alu.add, accum_out=RSr[:, col:col + 1])

        def stage_back(s, erange=None, parts=("p", "d")):
            """p1 row-sums + d1/d2 presence counts."""
            T, F, r = Tt[s], Ff[s], Rr[s]
            key = (s, "pscr")
            if key not in Rr:
                Rr[key] = mpool.tile([128, 2048], bf16, tag="pscr",
                                     name=f"p_{s}")
            pscr = Rr[key]
            need_m = {}
            for li in (1, 2):
                if CFG[f"d{li}"].get(s, "fused") == "B":
                    mk = (s, f"m{li}")
                    if mk not in Rr:
                        mp = mpool if li == 1 else rpool
                        Rr[mk] = mp.tile([128, 2048], bf16, tag=f"m{li}",
                                         name=f"m{li}_{s}")
                        Rr[(s, f"g{li}")] = mpool.tile(
                            [128, 2048], bf16, tag=f"g{li}", name=f"g{li}_{s}")
                    need_m[li] = (Rr[mk], Rr[(s, f"g{li}")])
            for e in ([None] if erange is None else erange):
                sl = slice(0, 2048) if e is None else eslice(0, e)
                col0 = s * 4 + (0 if e is None else e)
                s_div = (CFG["denom_mode"] == "div"
                         or CFG["denom"].get(s) == "div")
                if "p" in parts and s_div:
                    # p_l = f_l / (1+sadd) directly (TT divide, 2x bf16);
                    # row-sums of p2 land in RSr (no p0 identity needed)
                    nc.vector.tensor_tensor(pscr[:, sl], F[:, sl], r[:, sl],
                                            Alu.divide)
                    nc.vector.tensor_scalar(
                        junkB[:, sl], pscr[:, sl], 1.0, 0.0, Alu.mult,
                        Alu.add, accum_out=RS1[:, col0:col0 + 1])
                    nc.vector.tensor_tensor(
                        pscr[:, sl], F[:, 2048 + sl.start:2048 + sl.stop],
                        r[:, sl], Alu.divide)
                    nc.vector.tensor_scalar(
                        junkB[:, sl], pscr[:, sl], 1.0, 0.0, Alu.mult,
                        Alu.add, accum_out=RSr[:, col0:col0 + 1])
                elif "p" in parts:
                    nc.vector.tensor_tensor(pscr[:, sl], F[:, sl], r[:, sl],
                                            Alu.mult)
                    nc.vector.tensor_scalar(
                        junkB[:, sl], pscr[:, sl], 1.0, 0.0, Alu.mult,
                        Alu.add, accum_out=RS1[:, col0:col0 + 1])
                if "d" not in parts:
                    continue
                # own-label slice vs other-label slice per label
                for li in (1, 2):
                    own = 0 if li == 1 else 2048
                    oth = 2048 - own
                    if li in need_m:
                        m, g = need_m[li]
                        nc.vector.tensor_scalar_max(
                            m[:, sl], T[:, oth + sl.start:oth + sl.stop], 0.0)
                        nc.gpsimd.tensor_tensor(
                            g[:, sl], T[:, own + sl.start:own + sl.stop],
                            m[:, sl], Alu.subtract)
                es = range(NB) if e is None else (e,)
                DMs = {1: DM1, 2: DM2}
                for ee in es:
                    col = s * 4 + ee
                    for li in (1, 2):
                        own = 0 if li == 1 else 2048
                        oth = 2048 - own
                        jnk = junkA if li == 1 else junkB
                        if li in need_m:
                            _, g = need_m[li]
                            nc.vector.tensor_scalar(
                                jnk[:, eslice(0, ee)], g[:, eslice(0, ee)],
                                0.0, 0.0, Alu.is_gt, Alu.add,
                                accum_out=DMs[li][:, col:col + 1])
                        else:
                            nc.vector.scalar_tensor_tensor(
                                jnk[:, eslice(0, ee)], T[:, eslice(oth, ee)],
                                0.0, T[:, eslice(own, ee)], Alu.max,
                                Alu.is_lt, accum_out=DMs[li][:, col:col + 1])

        # ---- software-pipelined emission ----
        s_last = SPC - 1
        head_chunked = 0 in CHUNKED
        tail_chunked = s_last in CHUNKED

        def emit_loads(i, part):
            # part 'sp': SP-queue loads; part 'eng': Pool/ACT-queue loads
            if i == 0:
                if head_chunked:
                    na = CFG["act_chunks"]
                    if part == "sp":
                        for e in range(NB):
                            load_chunk(0, 0, e, "sp")
                        for e in range(na, NB):
                            load_chunk(0, 2, e, "sp")
                    else:
                        for e in range(NB):
                            load_chunk(0, 1, e, "pool")
                        for e in range(na):
                            load_chunk(0, 2, e, "act")
                else:
                    if part == "sp":
                        for c in range(C):
                            load_plane(0, c, "sp")
                return
            if i >= SPC:
                return
            if i == s_last and tail_chunked:
                if part == "sp" and not CFG["s7_interleave"]:
                    for c in range(C):
                        for e in range(NB):
                            load_chunk(s_last, c, e, "sp")
                return
            if i == SPC - 2 and CFG["weave_s6"]:
                return  # loaded via emit_s6_interleaved
            if i in CFG["spread_early"]:
                qmap = {0: "pool", 1: "sp", 2: "act"}
                for c in range(C):
                    if (part == "eng") == (qmap[c] != "sp"):
                        load_plane(i, c, qmap[c])
                return
            on_pool = i in CFG["pool_planes"]
            pc = CFG["pool_chan"]
            for c in range(C):
                is_pool = on_pool and c == pc
                is_eng = is_pool or (i, c) in CFG["act_planes"]
                if (part == "eng") == is_eng:
                    load_plane(i, c, "pool" if is_pool else "sp")

        def emit_s7_interleaved(i):
            # spread s7's 12 chunk loads over iterations 4..7 (3 per iter)
            if tail_chunked and CFG["s7_interleave"]:
                st = CFG["s7_il_start"]
                if st <= i <= st + 3:
                    e = i - st
                    for c in range(C):
                        load_chunk(s_last, c, e, "sp")
            if CFG["weave_s6"]:
                st = CFG["s6_il_start"]
                if st <= i <= st + 3:
                    e = i - st
                    for c in range(C):
                        load_chunk(SPC - 2, c, e, "sp")

        def mb_chunked(j):
            if j not in CHUNKED:
                return None
            if j == 0 and CFG["s0_front_only"]:
                return None
            return range(NB)

        d_in_mid = CFG["d_stage"] == "mid"

        def skipw(j):
            if CFG["weave_s7"] and tail_chunked and j == s_last:
                return True
            return CFG["weave_s6"] and j == SPC - 2

        def do_front(i):
            j = i - 2
            if 0 <= j < SPC and not skipw(j):
                er = range(NB) if j in CHUNKED else None
                stage_front(j, er)

        def do_mid(i):
            j = i - 3
            if 0 <= j < SPC and not skipw(j):
                stage_mid(j, mb_chunked(j))
                if d_in_mid:
                    stage_back(j, mb_chunked(j), parts=("d",))

        def do_back(i):
            j = i - CFG["back_lag"]
            if 0 <= j < SPC and not skipw(j):
                stage_back(j, mb_chunked(j),
                           parts=("p",) if d_in_mid else ("p", "d"))

        SMAP = dict(f=do_front, m=do_mid, b=do_back)
        weave = CFG["weave_s7"] and tail_chunked
        if CFG["dma_sub"]:
            weave = False

        wl = CFG["weave_lag"]

        def weave_one(i, s, lag):
            e = i - lag
            if 0 <= e < NB:
                stage_front(s, (e,))
            e = i - lag - 1
            if 0 <= e < NB:
                stage_mid(s, (e,))
                if d_in_mid:
                    stage_back(s, (e,), parts=("d",))
            e = i - lag - 2
            if 0 <= e < NB:
                if d_in_mid or not CFG["drain_d_first"]:
                    stage_back(s, (e,),
                               parts=("p",) if d_in_mid else ("p", "d"))
                else:
                    stage_back(s, (e,), parts=("d",))
                    stage_back(s, (e,), parts=("p",))

        def do_weave(i):
            if CFG["weave_s6"]:
                weave_one(i, SPC - 2, CFG["s6_lag"])
            if weave:
                weave_one(i, s_last, wl)

        if CFG["dma_sub"]:
            # x0 loads lead the accum DMAs by one sample so the Pool queue
            # never head-of-line blocks on its SP partner
            for i in range(SPC + 5):
                if i < SPC:
                    load_sub_base(i, 0)
                    load_sub_base(i, 1)
                j = i - 1
                if 0 <= j < SPC:
                    load_sub_accum(j, 0)
                    load_sub_accum(j, 1)
                j = i - 3
                if 0 <= j < SPC:
                    stage_front(j)
                j = i - 4
                if 0 <= j < SPC:
                    stage_mid(j)
                    if d_in_mid:
                        stage_back(j, parts=("d",))
                j = i - 5
                if 0 <= j < SPC:
                    stage_back(j, parts=("p",) if d_in_mid else ("p", "d"))
                if i == SPC - 1:
                    emit_consts()
        else:
         for i in range(SPC + 4):
             if CFG["load_pos"] == "split":
                 emit_loads(i, "sp")
                 emit_s7_interleaved(i)
             ordered = [SMAP[ch] for ch in CFG["stage_order"]]
             ordered[0](i)
             do_weave(i)
             if CFG["load_pos"] == "split":
                 emit_loads(i, "eng")
             for fn in ordered[1:]:
                 fn(i)
             if CFG["load_pos"] == "last":
                 emit_loads(i, "sp")
                 emit_loads(i, "eng")
                 emit_s7_interleaved(i)
             if i == SPC - 1 and CFG["consts_q"] in ("sp_late", "pool_late"):
                 emit_consts()

        # ---- tail ----
        O = cpool.tile([1, 40], f32, tag="O")
        S1 = ppool.tile([1, 64], f32, tag="S1")
        if CFG["tail_split"]:
            # samples 0-5 finalize ~10us before s6/s7: sum them early
            nc.tensor.matmul(S1[:, 0:24], ones[:, :], RS1[:, 0:24],
                             start=True, stop=True)
            nc.tensor.matmul(S1[:, 32:56], ones[:, :], RSr[:, 0:24],
                             start=True, stop=True)
            nc.tensor.matmul(S1[:, 24:32], ones[:, :], RS1[:, 24:32],
                             start=True, stop=True)
            nc.tensor.matmul(S1[:, 56:64], ones[:, :], RSr[:, 24:32],
                             start=True, stop=True)
        else:
            nc.tensor.matmul(S1[:, 0:32], ones[:, :], RS1[:, :],
                             start=True, stop=True)
            nc.tensor.matmul(S1[:, 32:64], ones[:, :], RSr[:, :],
                             start=True, stop=True)

        heights = []
        for li, DM in enumerate((DM1, DM2)):
            TD = ppool.tile([32, 128], f32, tag=f"TD{li}")
            nc.tensor.transpose(TD[:, :], DM[:, :], ident[:, :])
            pen = cpool.tile([32, 128], f32, tag=f"pen{li}")
            nc.vector.tensor_scalar(pen[:, :], TD[:, :], 0.5, 1e6,
                                    Alu.is_lt, Alu.mult)
            teng = nc.vector if CFG["tail_tt_eng"] == "dve" else nc.gpsimd
            cmin = cpool.tile([32, 128], f32, tag=f"cmin{li}")
            teng.tensor_tensor(cmin[:, :], pen[:, :], iota[:, :], Alu.add)
            cmax = cpool.tile([32, 128], f32, tag=f"cmax{li}")
            teng.tensor_tensor(cmax[:, :], iota[:, :], pen[:, :],
                               Alu.subtract)
            Y = cpool.tile([32, 2], f32, tag=f"Y{li}")
            nc.vector.tensor_reduce(Y[:, 0:1], cmin[:, :], X_AX, op=Alu.min)
            nc.vector.tensor_reduce(Y[:, 1:2], cmax[:, :], X_AX, op=Alu.max)
            YTmin = ppool.tile([1, 32], f32, tag=f"YTmin{li}")
            YTmax = ppool.tile([1, 32], f32, tag=f"YTmax{li}")
            nc.tensor.transpose(YTmin[:, :], Y[:, 0:1], ident[0:32, 0:32])
            nc.tensor.transpose(YTmax[:, :], Y[:, 1:2], ident[0:32, 0:32])
            ymin8 = cpool.tile([1, 8], f32, tag=f"ymin{li}")
            ymax8 = cpool.tile([1, 8], f32, tag=f"ymax{li}")
            nc.vector.tensor_reduce(
                ymin8[:, :],
                YTmin[0:1, :].rearrange("p (s e) -> p s e", e=4),
                X_AX, op=Alu.min)
            nc.vector.tensor_reduce(
                ymax8[:, :],
                YTmax[0:1, :].rearrange("p (s e) -> p s e", e=4),
                X_AX, op=Alu.max)
            hL = cpool.tile([1, 8], f32, tag=f"h{li}")
            nc.vector.tensor_tensor(hL[:, :], ymax8[:, :], ymin8[:, :],
                                    Alu.subtract)
            nc.vector.tensor_scalar_max(hL[:, :], hL[:, :], 0.0)
            heights.append(hL)

        h_cup, h_disc = heights
        den = cpool.tile([1, 8], f32, tag="den")
        nc.vector.tensor_scalar_add(den[:, :], h_disc[:, :], 1e-6)
        rec = cpool.tile([1, 8], f32, tag="rec")
        nc.vector.reciprocal(rec[:, :], den[:, :])
        nc.vector.tensor_tensor(O[:, 0:8], h_cup[:, :], rec[:, :], Alu.mult)

        s1tot = cpool.tile([1, 8], f32, tag="s1tot")
        srtot = cpool.tile([1, 8], f32, tag="srtot")
        p2tot = cpool.tile([1, 8], f32, tag="p2tot")
        p2a = cpool.tile([1, 8], f32, tag="p2a")
        sc = 1.0 / HW

        def mean_chain(lo, hi):
            n = hi - lo
            nc.vector.tensor_reduce(
                s1tot[:, lo:hi],
                S1[0:1, 4 * lo:4 * hi].rearrange("p (s e) -> p s e", e=4),
                X_AX, op=Alu.add)
            nc.vector.tensor_reduce(
                srtot[:, lo:hi],
                S1[0:1, 32 + 4 * lo:32 + 4 * hi].rearrange(
                    "p (s e) -> p s e", e=4),
                X_AX, op=Alu.add)
            if CFG["denom_mode"] == "div":
                nc.vector.tensor_copy(p2tot[:, lo:hi], srtot[:, lo:hi])
            else:
                nc.vector.tensor_scalar(p2a[:, lo:hi], srtot[:, lo:hi],
                                        -1.0, HW, Alu.mult, Alu.add)
                nc.vector.tensor_tensor(p2tot[:, lo:hi], p2a[:, lo:hi],
                                        s1tot[:, lo:hi], Alu.subtract)
                for s in range(lo, hi):
                    if CFG["denom"].get(s) == "div":
                        nc.vector.tensor_copy(p2tot[:, s:s + 1],
                                              srtot[:, s:s + 1])
            nc.vector.tensor_scalar_mul(O[:, 8 + lo:8 + hi],
                                        p2tot[:, lo:hi], sc)
            nc.vector.tensor_scalar_mul(O[:, 16 + lo:16 + hi],
                                        s1tot[:, lo:hi], sc)
            nc.vector.tensor_scalar_mul(O[:, 24 + lo:24 + hi],
                                        p2tot[:, lo:hi], sc)
            nc.vector.tensor_scalar_mul(O[:, 32 + lo:32 + hi],
                                        s1tot[:, lo:hi], sc)

        if CFG["tail_split"]:
            mean_chain(6, 8)
            mean_chain(0, 6)
        else:
            mean_chain(0, 8)

        nc.sync.dma_start(out[:, :], O[:, :])

    nc.finalize()
    return nc


def _get_nc():
    if "nc" not in _CACHE:
        _CACHE["nc"] = _build()
    return _CACHE["nc"]


def _host_inputs():
    iota = (np.arange(128, dtype=np.float32)[None, :]
            + 128.0 * np.tile(np.arange(4, dtype=np.float32), 8)[:, None])
    ident = np.eye(128, dtype=np.float32)
    ones = np.ones((128, 1), dtype=np.float32)
    return iota, ident, ones


def _run(seg_mask, trace=False):
    from concourse.bass_utils import run_bass_kernel_spmd

    x = np.ascontiguousarray(np.asarray(seg_mask, dtype=np.float32))
    assert x.shape == (B, C, H, W)
    iota, ident, ones = _host_inputs()
    in_maps = [
        {"x": x[SPC * c:SPC * (c + 1)], "iota": iota, "ident": ident,
         "ones": ones}
        for c in range(NCORES)
    ]
    nc = _get_nc()
    res = run_bass_kernel_spmd(nc, in_maps, core_ids=list(range(NCORES)),
                               trace=trace)
    outs = []
    for c in range(NCORES):
        o = np.asarray(res.results[c]["out"]).reshape(5, SPC).T
        outs.append(o)
    full = np.concatenate(outs, axis=0).astype(np.float32)
    return full, res


def kernel(segmentation_mask):
    full, _ = _run(segmentation_mask, trace=False)
    return full



# revision 19
# speedup vs baseline: 1.0986x; 1.0986x over previous
"""Trainium2 Bass kernel for CDRExtractor (segment_reduce).

Input : segmentation_mask (64, 3, 512, 512) fp32
Output: (64, 5) fp32 = [cdr, disc_mean, cup_mean, disc_mean, cup_mean]

Sharding: pure data parallel, 8 samples per core across 8 cores.

v3 design. Key ideas vs the 76.8us v2:
  - SWDGE (gpsimd) casting DMA loads fp32->bf16 at HALF the queue cost
    (cost model prices DMA by OUTPUT bytes per partition).  ~7 planes
    load as bf16 casts on the Pool queue; the other ~17 load fp32 on SP.
  - ACT only does exp(T) + ln/exp denominators for k samples;
    denominators for the rest go through DVE reciprocal.  Consts DMA +
    act-table warm hide in ACT's ramp.
  - Everything after exp works in f-space (exp is monotonic):
    d1 = rowcount[f1 > max(f2,1)], A = rowcount[max(f1,f2) > 1],
    d2 = A - d1 in the tail.  This kills the separate T tiles: subs
    write in place (cast) or into F, exp runs in place.
  - All accumulations (p1, sum_r, A, d1) are DVE tensor_scalar 4x or
    fused STT; Pool cannot run TensorScalar on real TRN2 (ISA).
Per-pixel math (bf16): t=[x1-x0|x2-x0], f=exp(t), sadd=f1+f2,
  r=1/(1+sadd) (ACT ln/exp+accum | DVE ts_add+recip+ts-accum),
  sum p1 = accum(f1*r), sum p2 = HW - sum r - sum p1.
"""

import numpy as np
from contextlib import ExitStack

B, C, H, W = 64, 3, 512, 512
NCORES = 8
SPC = B // NCORES      # samples per core = 8
NB = H // 128          # 128-row blocks = 4
HW = float(H * W)

_CACHE = {}

# ---- per-sample engine/config tables ----
CFG = dict(
    # load: 'cast' (Pool SWDGE bf16) | 'sp' (fp32 on SP) | 'mixed'
    # (x0,x1 fp32 on SP; x2 cast on Pool)
    load={0: "cast", 1: "sp", 2: "mixed", 3: "cast", 4: "mixed", 5: "sp",
          6: "mixed", 7: "sp"},
    denom={0: "dve", 1: "act", 2: "act", 3: "dve", 4: "act", 5: "split",
           6: "act", 7: "act"},
    subs={0: "dve", 1: "pool", 2: "pool", 3: "dve", 4: "pool",
          5: "pool", 6: "pool", 7: "pool"},
    sadd={0: "dve", 1: "pool", 2: "pool", 3: "dve", 4: "pool",
          5: "pool", 6: "pool", 7: "pool"},
    p1={0: "pool", 1: "dve", 2: "pool", 3: "pool", 4: "dve", 5: "dve",
        6: "pool", 7: "dve"},
    mEng={0: "dve", 1: "dve", 2: "dve", 3: "dve", 4: "dve", 5: "dve",
          6: "dve", 7: "dve"},
    # d1 mode: 'fused' (DVE STT 1x) | 'B' (DVE ts_max + g-sub TT + is_gt)
    d1={0: "fused", 1: "B", 2: "fused", 3: "fused", 4: "B", 5: "B",
        6: "B", 7: "B"},
    d1_g_eng={1: "pool", 4: "pool", 5: "pool", 6: "pool", 7: "pool"},
    lag_front=2, lag_mid=3, lag_back=4,
    chunk_head=True,     # s0 loads/front at half-plane granularity
    chunk_s1=True,       # s1 subs/exps at half granularity (ramp)
    weave=(5, 6, 7),     # samples staged at half granularity near the end
    weave_lag={5: 5, 6: 6, 7: 7},
    il_start={5: 3, 6: 4, 7: 5},   # iteration when woven loads interleave
    order="bmfl",        # emission order within an iteration
    tail_tt="pool",      # engine for tail TT ops
)


def _build():
    import concourse.bass as bass
    import concourse.bacc as bacc
    import concourse.mybir as mybir
    from concourse.tile import TileContext

    if not _CACHE.get("act_patch"):
        _orig_tables = bacc.get_activation_tables

        def _only_ln_exp(arch):
            t = _orig_tables(arch)
            keep = "natural_log_exp_and_others"
            return {k: (v if k == keep else set()) for k, v in t.items()}

        bacc.get_activation_tables = _only_ln_exp
        _CACHE["act_patch"] = True

    f32 = mybir.dt.float32
    bf16 = mybir.dt.bfloat16
    Alu = mybir.AluOpType
    AFT = mybir.ActivationFunctionType
    X_AX = mybir.AxisListType.X

    nc = bacc.Bacc()
    x = nc.dram_tensor("x", (SPC, C, H, W), f32, kind="ExternalInput")
    iota_in = nc.dram_tensor("iota", (32, 128), f32, kind="ExternalInput")
    ident_in = nc.dram_tensor("ident", (128, 128), f32, kind="ExternalInput")
    ones_in = nc.dram_tensor("ones", (128, 1), f32, kind="ExternalInput")
    out = nc.dram_tensor("out", (5, SPC), f32, kind="ExternalOutput")

    s_last = SPC - 1

    def is_cast(s):
        return CFG["load"][s] == "cast"

    with TileContext(nc) as tc, ExitStack() as ctx:
        QENG = dict(sp=nc.sync, act=nc.scalar, pool=nc.gpsimd,
                    dve=nc.vector)
        cpool = ctx.enter_context(tc.tile_pool(name="consts", bufs=1))
        apool = ctx.enter_context(tc.tile_pool(name="accs", bufs=1))
        xcpool = ctx.enter_context(tc.tile_pool(name="xc", bufs=3))
        xfpool = ctx.enter_context(tc.tile_pool(name="xf", bufs=2))
        fpool = ctx.enter_context(tc.tile_pool(name="fmain", bufs=3))
        mpool = ctx.enter_context(tc.tile_pool(name="mid", bufs=3))
        bpool = ctx.enter_context(tc.tile_pool(name="bck", bufs=2))
        rpool = ctx.enter_context(tc.tile_pool(name="rr", bufs=3))
        ppool = ctx.enter_context(tc.tile_pool(name="ps", bufs=1,
                                               space="PSUM"))

        # act-table warm + consts ride the ACT queue's idle ramp
        warm = cpool.tile([1, 16], bf16, tag="warm")
        nc.vector.memset(warm[:, :], 0.0)
        nc.scalar.activation(warm[:, :], warm[:, :], AFT.Exp)

        iota = cpool.tile([32, 128], f32, tag="iota")
        ident = cpool.tile([128, 128], f32, tag="ident")
        ones = cpool.tile([128, 1], f32, tag="ones")
        nc.scalar.dma_start(iota[:, :], iota_in[:, :])
        nc.scalar.dma_start(ident[:, :], ident_in[:, :])
        nc.scalar.dma_start(ones[:, :], ones_in[:, :])

        # accumulators: col j = s*4 + e  (e = h//128 block)
        RS1 = apool.tile([128, 32], f32, tag="RS1")   # sum p1
        RSr = apool.tile([128, 32], f32, tag="RSr")   # sum r
        DM1 = apool.tile([128, 32], f32, tag="DM1")   # rowcount argmax==1
        DMA_ = apool.tile([128, 32], f32, tag="DMA")  # rowcount argmax in {1,2}
        for acc_t in (RS1, RSr, DM1, DMA_):
            nc.vector.memset(acc_t[:, :], 0.0)

        junkA = cpool.tile([128, 2048], bf16, tag="junkA")
        junkB = cpool.tile([128, 2048], bf16, tag="junkB")

        X = {}
        Ff = {}
        Rr = {}

        def eslice(base, e):
            return slice(base + e * 512, base + (e + 1) * 512)

        def esl(e):
            return slice(0, 2048) if e is None else eslice(0, e)

        def ld(s, c):
            mode = CFG["load"][s]
            if mode == "cast":
                return "pool", bf16
            if mode == "sp":
                return "sp", f32
            return ("pool", bf16) if c == 2 else ("sp", f32)

        def load_plane(s, c, half=None):
            q, dt = ld(s, c)
            key = (s, c)
            if key not in X:
                pool_ = xcpool if dt == bf16 else xfpool
                X[key] = pool_.tile([128, NB, 512], dt,
                                    tag=f"X{'c' if dt == bf16 else 'f'}{c}",
                                    name=f"X_{s}_{c}")
            if half is None:
                src = x[s, c].rearrange("(e p) w -> p e w", p=128)
                QENG[q].dma_start(X[key], src)
            else:
                e0 = half * 2
                src = x[s, c, e0 * 128:(e0 + 2) * 128, :].rearrange(
                    "(e p) w -> p e w", p=128)
                QENG[q].dma_start(X[key][:, e0:e0 + 2], src)

        def fview(s, li):
            """AP of f_l (exp of t_l) as (128, 2048)."""
            if is_cast(s):
                return X[(s, li)].rearrange("p e w -> p (e w)")
            return Ff[s][:, (li - 1) * 2048:li * 2048]

        def csl(ch):
            if ch is None:
                return slice(0, 2048)
            e0, ne = ch
            return slice(e0 * 512, (e0 + ne) * 512)

        def stage_front(s, chunks=(None,)):
            """t halves built (in place for cast), then f = exp(t) in place."""
            cast = is_cast(s)
            if not cast and s not in Ff:
                Ff[s] = fpool.tile([128, 4096], bf16, tag="F", name=f"F_{s}")
            se = QENG[CFG["subs"][s]]
            x0 = X[(s, 0)].rearrange("p e w -> p (e w)")
            x1 = X[(s, 1)].rearrange("p e w -> p (e w)")
            x2 = X[(s, 2)].rearrange("p e w -> p (e w)")
            t1 = fview(s, 1)
            t2 = fview(s, 2)
            for ch in chunks:
                sl = csl(ch)
                se.tensor_tensor(t1[:, sl], x1[:, sl], x0[:, sl],
                                 Alu.subtract)
                se.tensor_tensor(t2[:, sl], x2[:, sl], x0[:, sl],
                                 Alu.subtract)
                if cast or ch is not None:
                    nc.scalar.activation(t1[:, sl], t1[:, sl], AFT.Exp)
                    nc.scalar.activation(t2[:, sl], t2[:, sl], AFT.Exp)
                else:
                    F = Ff[s]
                    nc.scalar.activation(F[:, :], F[:, :], AFT.Exp)

        def stage_mid(s, chunks=(None,)):
            """sadd = f1+f2; denominator r (+ sum r accumulated)."""
            if s not in Rr:
                Rr[s] = rpool.tile([128, 2048], bf16, tag="r", name=f"r_{s}")
                Rr[(s, "sadd")] = mpool.tile([128, 2048], bf16, tag="sadd",
                                             name=f"sa_{s}")
                Rr[(s, "aux")] = mpool.tile([128, 2048], bf16, tag="aux",
                                            name=f"aux_{s}")
            r, sadd, aux = Rr[s], Rr[(s, "sadd")], Rr[(s, "aux")]
            f1 = fview(s, 1)
            f2 = fview(s, 2)
            saddf = QENG[CFG["sadd"][s]]
            dmode = CFG["denom"][s]
            if dmode == "split" and chunks == (None,):
                chunks = [(0, 2), (2, 2)]
            for ch in chunks:
                sl = csl(ch)
                col = s * 4 + (0 if ch is None else ch[0])
                saddf.tensor_tensor(sadd[:, sl], f1[:, sl], f2[:, sl],
                                    Alu.add)
                dve_denom = (dmode == "dve"
                             or (dmode == "split" and ch is not None
                                 and ch[0] >= 2))
                if dve_denom:
                    nc.vector.tensor_scalar_add(aux[:, sl], sadd[:, sl], 1.0)
                    with nc.allow_low_precision(reason="bf16 softmax denom"):
                        nc.vector.reciprocal(r[:, sl], aux[:, sl])
                    nc.vector.tensor_scalar(
                        junkA[:, sl], r[:, sl], 1.0, 0.0, Alu.mult, Alu.add,
                        accum_out=RSr[:, col:col + 1])
                else:
                    nc.scalar.activation(aux[:, sl], sadd[:, sl], AFT.Ln,
                                         bias=1.0)
                    nc.scalar.activation(r[:, sl], aux[:, sl], AFT.Exp,
                                         scale=-1.0,
                                         accum_out=RSr[:, col:col + 1])

        def stage_back(s, chunks=(None,)):
            """p1 sums; m = max(f1,f2) + A counts; d1 counts (f-space)."""
            r = Rr[s]
            f1 = fview(s, 1)
            f2 = fview(s, 2)
            if (s, "pscr") not in Rr:
                Rr[(s, "pscr")] = bpool.tile([128, 2048], bf16, tag="pscr",
                                             name=f"p_{s}")
                Rr[(s, "mm")] = rpool.tile([128, 2048], bf16, tag="mm",
                                           name=f"mm_{s}")
            pscr, mm = Rr[(s, "pscr")], Rr[(s, "mm")]
            p1f = QENG[CFG["p1"][s]]
            mmf = QENG[CFG["mEng"][s]]
            d1_mode = CFG["d1"][s]
            if d1_mode == "B" and (s, "m1") not in Rr:
                Rr[(s, "m1")] = bpool.tile([128, 2048], bf16, tag="m1",
                                           name=f"m1_{s}")
                Rr[(s, "g1")] = bpool.tile([128, 2048], bf16, tag="g1",
                                           name=f"g1_{s}")
            for ch in chunks:
                sl = csl(ch)
                col0 = s * 4 + (0 if ch is None else ch[0])
                # p1 = f1 * r, row-sums into RS1
                p1f.tensor_tensor(pscr[:, sl], f1[:, sl], r[:, sl], Alu.mult)
                nc.vector.tensor_scalar(
                    junkB[:, sl], pscr[:, sl], 1.0, 0.0, Alu.mult, Alu.add,
                    accum_out=RS1[:, col0:col0 + 1])
                # m = max(f1, f2)
                mmf.tensor_tensor(mm[:, sl], f1[:, sl], f2[:, sl], Alu.max)
                if d1_mode == "B":
                    m1, g1 = Rr[(s, "m1")], Rr[(s, "g1")]
                    nc.vector.tensor_scalar_max(m1[:, sl], f2[:, sl], 1.0)
                    ge = QENG[CFG["d1_g_eng"].get(s, "pool")]
                    ge.tensor_tensor(g1[:, sl], f1[:, sl], m1[:, sl],
                                     Alu.subtract)
                es = (range(NB) if ch is None
                      else range(ch[0], ch[0] + ch[1]))
                for ee in es:
                    col = s * 4 + ee
                    # A = rowcount[max(f1,f2) > 1]
                    nc.vector.tensor_scalar(
                        junkA[:, eslice(0, ee)], mm[:, eslice(0, ee)],
                        1.0, 0.0, Alu.is_gt, Alu.add,
                        accum_out=DMA_[:, col:col + 1])
                    # d1 = rowcount[max(f2,1) < f1]
                    if d1_mode == "B":
                        nc.vector.tensor_scalar(
                            junkB[:, eslice(0, ee)],
                            Rr[(s, "g1")][:, eslice(0, ee)],
                            0.0, 0.0, Alu.is_gt, Alu.add,
                            accum_out=DM1[:, col:col + 1])
                    else:
                        nc.vector.scalar_tensor_tensor(
                            junkB[:, eslice(0, ee)], f2[:, eslice(0, ee)],
                            1.0, f1[:, eslice(0, ee)], Alu.max, Alu.is_lt,
                            accum_out=DM1[:, col:col + 1])

        # ---- software-pipelined emission ----
        WOVEN = set(CFG["weave"])

        def emit_loads(i):
            if i >= SPC:
                return
            if i == 0 and CFG["chunk_head"]:
                for half in (0, 1):
                    for c in range(C):
                        load_plane(0, c, half=half)
                return
            if i in WOVEN:
                return  # interleaved below
            for c in range(C):
                load_plane(i, c)

        def emit_woven_loads(i):
            for s in sorted(WOVEN):
                st = CFG["il_start"][s]
                if st <= i <= st + 1:
                    h = i - st
                    for c in range(C):
                        load_plane(s, c, half=h)

        def chunked(j, stage):
            if j == 0 and CFG["chunk_head"]:
                return [(0, 1), (1, 1), (2, 2)]
            if j == 1 and CFG["chunk_s1"] and stage == "f":
                return [(0, 2), (2, 2)]
            return (None,)

        def do_front(i):
            j = i - CFG["lag_front"]
            if 0 <= j < SPC and j not in WOVEN:
                stage_front(j, chunked(j, "f"))

        def do_mid(i):
            j = i - CFG["lag_mid"]
            if 0 <= j < SPC and j not in WOVEN:
                stage_mid(j, chunked(j, "m"))

        def do_back(i):
            j = i - CFG["lag_back"]
            if 0 <= j < SPC and j not in WOVEN:
                stage_back(j, chunked(j, "b"))

        # woven samples run half-granular front/mid, half-granular back
        WCH = [(0, 2), (2, 2)]

        def do_weave(i):
            for s in sorted(WOVEN):
                wl = CFG["weave_lag"][s]
                h = i - wl
                if 0 <= h < 2:
                    stage_front(s, (WCH[h],))
                h = i - wl - 1
                if 0 <= h < 2:
                    stage_mid(s, (WCH[h],))
                h = i - wl - 2
                if 0 <= h < 2:
                    stage_back(s, (WCH[h],))

        n_iter = max([SPC + CFG["lag_back"] + 1]
                     + [CFG["weave_lag"][s] + 2 + 3 for s in WOVEN])
        OMAP = {"b": do_back, "m": do_mid, "f": do_front}
        for i in range(n_iter):
            for ch in CFG["order"]:
                if ch == "l":
                    emit_loads(i)
                    emit_woven_loads(i)
                else:
                    OMAP[ch](i)
            do_weave(i)

        # ---- tail ----
        O = cpool.tile([1, 40], f32, tag="O")
        S1 = ppool.tile([1, 64], f32, tag="S1")
        nc.tensor.matmul(S1[:, 0:32], ones[:, :], RS1[:, :],
                         start=True, stop=True)
        nc.tensor.matmul(S1[:, 32:64], ones[:, :], RSr[:, :],
                         start=True, stop=True)

        # DM2 = A - DM1
        teng = QENG[CFG.get("tail_tt", "dve")]
        DM2 = apool.tile([128, 32], f32, tag="DM2")
        teng.tensor_tensor(DM2[:, :], DMA_[:, :], DM1[:, :],
                           Alu.subtract)

        heights = []
        for li, DM in enumerate((DM1, DM2)):
            TD = ppool.tile([32, 128], f32, tag=f"TD{li}")
            nc.tensor.transpose(TD[:, :], DM[:, :], ident[:, :])
            pen = cpool.tile([32, 128], f32, tag=f"pen{li}")
            nc.vector.tensor_scalar(pen[:, :], TD[:, :], 0.5, 1e6,
                                    Alu.is_lt, Alu.mult)
            cmin = cpool.tile([32, 128], f32, tag=f"cmin{li}")
            teng.tensor_tensor(cmin[:, :], pen[:, :], iota[:, :],
                               Alu.add)
            cmax = cpool.tile([32, 128], f32, tag=f"cmax{li}")
            teng.tensor_tensor(cmax[:, :], iota[:, :], pen[:, :],
                               Alu.subtract)
            Y = cpool.tile([32, 2], f32, tag=f"Y{li}")
            nc.vector.tensor_reduce(Y[:, 0:1], cmin[:, :], X_AX, op=Alu.min)
            nc.vector.tensor_reduce(Y[:, 1:2], cmax[:, :], X_AX, op=Alu.max)
            YTmin = ppool.tile([1, 32], f32, tag=f"YTmin{li}")
            YTmax = ppool.tile([1, 32], f32, tag=f"YTmax{li}")
            nc.tensor.transpose(YTmin[:, :], Y[:, 0:1], ident[0:32, 0:32])
            nc.tensor.transpose(YTmax[:, :], Y[:, 1:2], ident[0:32, 0:32])
            ymin8 = cpool.tile([1, 8], f32, tag=f"ymin{li}")
            ymax8 = cpool.tile([1, 8], f32, tag=f"ymax{li}")
            nc.vector.tensor_reduce(
                ymin8[:, :],
                YTmin[0:1, :].rearrange("p (s e) -> p s e", e=4),
                X_AX, op=Alu.min)
            nc.vector.tensor_reduce(
                ymax8[:, :],
                YTmax[0:1, :].rearrange("p (s e) -> p s e", e=4),
                X_AX, op=Alu.max)
            hL = cpool.tile([1, 8], f32, tag=f"h{li}")
            nc.vector.tensor_tensor(hL[:, :], ymax8[:, :], ymin8[:, :],
                                    Alu.subtract)
            nc.vector.tensor_scalar_max(hL[:, :], hL[:, :], 0.0)
            heights.append(hL)

        h_cup, h_disc = heights
        den = cpool.tile([1, 8], f32, tag="den")
        nc.vector.tensor_scalar_add(den[:, :], h_disc[:, :], 1e-6)
        rec = cpool.tile([1, 8], f32, tag="rec")
        nc.vector.reciprocal(rec[:, :], den[:, :])
        nc.vector.tensor_tensor(O[:, 0:8], h_cup[:, :], rec[:, :], Alu.mult)

        s1tot = cpool.tile([1, 8], f32, tag="s1tot")
        srtot = cpool.tile([1, 8], f32, tag="srtot")
        p2tot = cpool.tile([1, 8], f32, tag="p2tot")
        p2a = cpool.tile([1, 8], f32, tag="p2a")
        sc = 1.0 / HW

        nc.vector.tensor_reduce(
            s1tot[:, :],
            S1[0:1, 0:32].rearrange("p (s e) -> p s e", e=4),
            X_AX, op=Alu.add)
        nc.vector.tensor_reduce(
            srtot[:, :],
            S1[0:1, 32:64].rearrange("p (s e) -> p s e", e=4),
            X_AX, op=Alu.add)
        nc.vector.tensor_scalar(p2a[:, :], srtot[:, :], -1.0, HW,
                                Alu.mult, Alu.add)
        nc.vector.tensor_tensor(p2tot[:, :], p2a[:, :], s1tot[:, :],
                                Alu.subtract)
        nc.vector.tensor_scalar_mul(O[:, 8:16], p2tot[:, :], sc)
        nc.vector.tensor_scalar_mul(O[:, 16:24], s1tot[:, :], sc)
        nc.vector.tensor_scalar_mul(O[:, 24:32], p2tot[:, :], sc)
        nc.vector.tensor_scalar_mul(O[:, 32:40], s1tot[:, :], sc)

        nc.sync.dma_start(out[:, :], O[:, :])

    nc.finalize()
    return nc


def _get_nc():
    if "nc" not in _CACHE:
        _CACHE["nc"] = _build()
    return _CACHE["nc"]


def _host_inputs():
    iota = (np.arange(128, dtype=np.float32)[None, :]
            + 128.0 * np.tile(np.arange(4, dtype=np.float32), 8)[:, None])
    ident = np.eye(128, dtype=np.float32)
    ones = np.ones((128, 1), dtype=np.float32)
    return iota, ident, ones


def _run(seg_mask, trace=False):
    from concourse.bass_utils import run_bass_kernel_spmd

    x = np.ascontiguousarray(np.asarray(seg_mask, dtype=np.float32))
    assert x.shape == (B, C, H, W)
    iota, ident, ones = _host_inputs()
    in_maps = [
        {"x": x[SPC * c:SPC * (c + 1)], "iota": iota, "ident": ident,
         "ones": ones}
        for c in range(NCORES)
    ]
    nc = _get_nc()
    res = run_bass_kernel_spmd(nc, in_maps, core_ids=list(range(NCORES)),
                               trace=trace)
    outs = []
    for c in range(NCORES):
        o = np.asarray(res.results[c]["out"]).reshape(5, SPC).T
        outs.append(o)
    full = np.concatenate(outs, axis=0).astype(np.float32)
    return full, res


def kernel(segmentation_mask):
    full, _ = _run(segmentation_mask, trace=False)
    return full


# revision 22
# speedup vs baseline: 1.1230x; 1.0223x over previous
"""Trainium2 Bass kernel for CDRExtractor (segment_reduce).

Input : segmentation_mask (64, 3, 512, 512) fp32
Output: (64, 5) fp32 = [cdr, disc_mean, cup_mean, disc_mean, cup_mean]

Sharding: pure data parallel, 8 samples per core across 8 cores.

v3 design. Key ideas vs the 76.8us v2:
  - SWDGE (gpsimd) casting DMA loads fp32->bf16 at HALF the queue cost
    (cost model prices DMA by OUTPUT bytes per partition).  ~7 planes
    load as bf16 casts on the Pool queue; the other ~17 load fp32 on SP.
  - ACT only does exp(T) + ln/exp denominators for k samples;
    denominators for the rest go through DVE reciprocal.  Consts DMA +
    act-table warm hide in ACT's ramp.
  - Everything after exp works in f-space (exp is monotonic):
    d1 = rowcount[f1 > max(f2,1)], A = rowcount[max(f1,f2) > 1],
    d2 = A - d1 in the tail.  This kills the separate T tiles: subs
    write in place (cast) or into F, exp runs in place.
  - All accumulations (p1, sum_r, A, d1) are DVE tensor_scalar 4x or
    fused STT; Pool cannot run TensorScalar on real TRN2 (ISA).
Per-pixel math (bf16): t=[x1-x0|x2-x0], f=exp(t), sadd=f1+f2,
  r=1/(1+sadd) (ACT ln/exp+accum | DVE ts_add+recip+ts-accum),
  sum p1 = accum(f1*r), sum p2 = HW - sum r - sum p1.
"""

import numpy as np
from contextlib import ExitStack

B, C, H, W = 64, 3, 512, 512
NCORES = 8
SPC = B // NCORES      # samples per core = 8
NB = H // 128          # 128-row blocks = 4
HW = float(H * W)

_CACHE = {}

# ---- per-sample engine/config tables ----
CFG = dict(
    # load: 'cast' (Pool SWDGE bf16) | 'sp' (fp32 on SP) | 'mixed'
    # (x0,x1 fp32 on SP; x2 cast on Pool)
    load={0: "cast", 1: "sp", 2: "mixed", 3: "cast", 4: "mixed", 5: "sp",
          6: "mixed", 7: "sp"},
    denom={0: "dve", 1: "act", 2: "act", 3: "dve", 4: "act", 5: "split",
           6: "act", 7: "act"},
    subs={0: "dve", 1: "pool", 2: "pool", 3: "dve", 4: "pool",
          5: "pool", 6: "pool", 7: "pool"},
    sadd={0: "dve", 1: "pool", 2: "pool", 3: "dve", 4: "pool",
          5: "pool", 6: "pool", 7: "pool"},
    p1={0: "pool", 1: "dve", 2: "pool", 3: "pool", 4: "dve", 5: "dve",
        6: "pool", 7: "dve"},
    mEng={0: "dve", 1: "dve", 2: "dve", 3: "dve", 4: "dve", 5: "dve",
          6: "dve", 7: "dve"},
    # d1 mode: 'fused' (DVE STT 1x) | 'B' (DVE ts_max + g-sub TT + is_gt)
    d1={0: "fused", 1: "B", 2: "fused", 3: "fused", 4: "B", 5: "B",
        6: "B", 7: "B"},
    d1_g_eng={1: "pool", 4: "pool", 5: "pool", 6: "pool", 7: "pool"},
    lag_front=2, lag_mid=3, lag_back=4,
    chunk_head=True,     # s0 loads/front at half-plane granularity
    chunk_s1=True,       # s1 subs/exps at half granularity (ramp)
    weave=(5, 6, 7),     # samples staged at half granularity near the end
    weave_lag={5: 5, 6: 6, 7: 7},
    il_start={5: 3, 6: 4, 7: 5},   # iteration when woven loads interleave
    order="bmfl",        # emission order within an iteration
    tail_tt="pool",      # engine for tail TT ops
)


def _build():
    import concourse.bass as bass
    import concourse.bacc as bacc
    import concourse.mybir as mybir
    from concourse.tile import TileContext

    if not _CACHE.get("act_patch"):
        _orig_tables = bacc.get_activation_tables

        def _only_ln_exp(arch):
            t = _orig_tables(arch)
            keep = "natural_log_exp_and_others"
            return {k: (v if k == keep else set()) for k, v in t.items()}

        bacc.get_activation_tables = _only_ln_exp
        _CACHE["act_patch"] = True

    f32 = mybir.dt.float32
    bf16 = mybir.dt.bfloat16
    Alu = mybir.AluOpType
    AFT = mybir.ActivationFunctionType
    X_AX = mybir.AxisListType.X
    C_AX = mybir.AxisListType.C

    nc = bacc.Bacc()
    x = nc.dram_tensor("x", (SPC, C, H, W), f32, kind="ExternalInput")
    iota_in = nc.dram_tensor("iota", (128, 64), f32, kind="ExternalInput")
    out = nc.dram_tensor("out", (5, SPC), f32, kind="ExternalOutput")

    s_last = SPC - 1

    def is_cast(s):
        return CFG["load"][s] == "cast"

    with TileContext(nc) as tc, ExitStack() as ctx:
        QENG = dict(sp=nc.sync, act=nc.scalar, pool=nc.gpsimd,
                    dve=nc.vector)
        cpool = ctx.enter_context(tc.tile_pool(name="consts", bufs=1))
        apool = ctx.enter_context(tc.tile_pool(name="accs", bufs=1))
        xcpool = ctx.enter_context(tc.tile_pool(name="xc", bufs=3))
        xfpool = ctx.enter_context(tc.tile_pool(name="xf", bufs=2))
        fpool = ctx.enter_context(tc.tile_pool(name="fmain", bufs=3))
        mpool = ctx.enter_context(tc.tile_pool(name="mid", bufs=3))
        bpool = ctx.enter_context(tc.tile_pool(name="bck", bufs=2))
        rpool = ctx.enter_context(tc.tile_pool(name="rr", bufs=3))

        # act-table warm + consts ride the ACT queue's idle ramp
        warm = cpool.tile([1, 16], bf16, tag="warm")
        nc.vector.memset(warm[:, :], 0.0)
        nc.scalar.activation(warm[:, :], warm[:, :], AFT.Exp)

        iota = cpool.tile([128, 64], f32, tag="iota")
        nc.scalar.dma_start(iota[:, :], iota_in[:, :])

        # accumulators: col j = s*4 + e  (e = h//128 block)
        RS1 = apool.tile([128, 32], f32, tag="RS1")   # sum p1
        RSr = apool.tile([128, 32], f32, tag="RSr")   # sum r
        DM1 = apool.tile([128, 32], f32, tag="DM1")   # rowcount argmax==1
        DMA_ = apool.tile([128, 32], f32, tag="DMA")  # rowcount argmax in {1,2}
        for acc_t in (RS1, RSr, DM1, DMA_):
            nc.vector.memset(acc_t[:, :], 0.0)

        junkA = cpool.tile([128, 2048], bf16, tag="junkA")
        junkB = cpool.tile([128, 2048], bf16, tag="junkB")

        X = {}
        Ff = {}
        Rr = {}

        def eslice(base, e):
            return slice(base + e * 512, base + (e + 1) * 512)

        def esl(e):
            return slice(0, 2048) if e is None else eslice(0, e)

        def ld(s, c):
            mode = CFG["load"][s]
            if mode == "cast":
                return "pool", bf16
            if mode == "sp":
                return "sp", f32
            return ("pool", bf16) if c == 2 else ("sp", f32)

        def load_plane(s, c, half=None):
            q, dt = ld(s, c)
            key = (s, c)
            if key not in X:
                pool_ = xcpool if dt == bf16 else xfpool
                X[key] = pool_.tile([128, NB, 512], dt,
                                    tag=f"X{'c' if dt == bf16 else 'f'}{c}",
                                    name=f"X_{s}_{c}")
            if half is None:
                src = x[s, c].rearrange("(e p) w -> p e w", p=128)
                QENG[q].dma_start(X[key], src)
            else:
                e0 = half * 2
                src = x[s, c, e0 * 128:(e0 + 2) * 128, :].rearrange(
                    "(e p) w -> p e w", p=128)
                QENG[q].dma_start(X[key][:, e0:e0 + 2], src)

        def fview(s, li):
            """AP of f_l (exp of t_l) as (128, 2048)."""
            if is_cast(s):
                return X[(s, li)].rearrange("p e w -> p (e w)")
            return Ff[s][:, (li - 1) * 2048:li * 2048]

        def csl(ch):
            if ch is None:
                return slice(0, 2048)
            e0, ne = ch
            return slice(e0 * 512, (e0 + ne) * 512)

        def stage_front(s, chunks=(None,)):
            """t halves built (in place for cast), then f = exp(t) in place."""
            cast = is_cast(s)
            if not cast and s not in Ff:
                Ff[s] = fpool.tile([128, 4096], bf16, tag="F", name=f"F_{s}")
            se = QENG[CFG["subs"][s]]
            x0 = X[(s, 0)].rearrange("p e w -> p (e w)")
            x1 = X[(s, 1)].rearrange("p e w -> p (e w)")
            x2 = X[(s, 2)].rearrange("p e w -> p (e w)")
            t1 = fview(s, 1)
            t2 = fview(s, 2)
            for ch in chunks:
                sl = csl(ch)
                se.tensor_tensor(t1[:, sl], x1[:, sl], x0[:, sl],
                                 Alu.subtract)
                se.tensor_tensor(t2[:, sl], x2[:, sl], x0[:, sl],
                                 Alu.subtract)
                if cast or ch is not None:
                    nc.scalar.activation(t1[:, sl], t1[:, sl], AFT.Exp)
                    nc.scalar.activation(t2[:, sl], t2[:, sl], AFT.Exp)
                else:
                    F = Ff[s]
                    nc.scalar.activation(F[:, :], F[:, :], AFT.Exp)

        def stage_mid(s, chunks=(None,)):
            """sadd = f1+f2; denominator r (+ sum r accumulated)."""
            if s not in Rr:
                Rr[s] = rpool.tile([128, 2048], bf16, tag="r", name=f"r_{s}")
                Rr[(s, "sadd")] = mpool.tile([128, 2048], bf16, tag="sadd",
                                             name=f"sa_{s}")
                Rr[(s, "aux")] = mpool.tile([128, 2048], bf16, tag="aux",
                                            name=f"aux_{s}")
            r, sadd, aux = Rr[s], Rr[(s, "sadd")], Rr[(s, "aux")]
            f1 = fview(s, 1)
            f2 = fview(s, 2)
            saddf = QENG[CFG["sadd"][s]]
            dmode = CFG["denom"][s]
            if dmode == "split" and chunks == (None,):
                chunks = [(0, 2), (2, 2)]
            for ch in chunks:
                sl = csl(ch)
                col = s * 4 + (0 if ch is None else ch[0])
                saddf.tensor_tensor(sadd[:, sl], f1[:, sl], f2[:, sl],
                                    Alu.add)
                dve_denom = (dmode == "dve"
                             or (dmode == "split" and ch is not None
                                 and ch[0] >= 2))
                if dve_denom:
                    nc.vector.tensor_scalar_add(aux[:, sl], sadd[:, sl], 1.0)
                    with nc.allow_low_precision(reason="bf16 softmax denom"):
                        nc.vector.reciprocal(r[:, sl], aux[:, sl])
                    nc.vector.tensor_scalar(
                        junkA[:, sl], r[:, sl], 1.0, 0.0, Alu.mult, Alu.add,
                        accum_out=RSr[:, col:col + 1])
                else:
                    nc.scalar.activation(aux[:, sl], sadd[:, sl], AFT.Ln,
                                         bias=1.0)
                    nc.scalar.activation(r[:, sl], aux[:, sl], AFT.Exp,
                                         scale=-1.0,
                                         accum_out=RSr[:, col:col + 1])

        def stage_back(s, chunks=(None,)):
            """p1 sums; m = max(f1,f2) + A counts; d1 counts (f-space)."""
            r = Rr[s]
            f1 = fview(s, 1)
            f2 = fview(s, 2)
            if (s, "pscr") not in Rr:
                Rr[(s, "pscr")] = bpool.tile([128, 2048], bf16, tag="pscr",
                                             name=f"p_{s}")
                Rr[(s, "mm")] = rpool.tile([128, 2048], bf16, tag="mm",
                                           name=f"mm_{s}")
            pscr, mm = Rr[(s, "pscr")], Rr[(s, "mm")]
            p1f = QENG[CFG["p1"][s]]
            mmf = QENG[CFG["mEng"][s]]
            d1_mode = CFG["d1"][s]
            if d1_mode == "B" and (s, "m1") not in Rr:
                Rr[(s, "m1")] = bpool.tile([128, 2048], bf16, tag="m1",
                                           name=f"m1_{s}")
                Rr[(s, "g1")] = bpool.tile([128, 2048], bf16, tag="g1",
                                           name=f"g1_{s}")
            for ch in chunks:
                sl = csl(ch)
                col0 = s * 4 + (0 if ch is None else ch[0])
                # p1 = f1 * r, row-sums into RS1
                p1f.tensor_tensor(pscr[:, sl], f1[:, sl], r[:, sl], Alu.mult)
                nc.vector.tensor_scalar(
                    junkB[:, sl], pscr[:, sl], 1.0, 0.0, Alu.mult, Alu.add,
                    accum_out=RS1[:, col0:col0 + 1])
                # m = max(f1, f2)
                mmf.tensor_tensor(mm[:, sl], f1[:, sl], f2[:, sl], Alu.max)
                if d1_mode == "B":
                    m1, g1 = Rr[(s, "m1")], Rr[(s, "g1")]
                    nc.vector.tensor_scalar_max(m1[:, sl], f2[:, sl], 1.0)
                    ge = QENG[CFG["d1_g_eng"].get(s, "pool")]
                    ge.tensor_tensor(g1[:, sl], f1[:, sl], m1[:, sl],
                                     Alu.subtract)
                es = (range(NB) if ch is None
                      else range(ch[0], ch[0] + ch[1]))
                for ee in es:
                    col = s * 4 + ee
                    # A = rowcount[max(f1,f2) > 1]
                    nc.vector.tensor_scalar(
                        junkA[:, eslice(0, ee)], mm[:, eslice(0, ee)],
                        1.0, 0.0, Alu.is_gt, Alu.add,
                        accum_out=DMA_[:, col:col + 1])
                    # d1 = rowcount[max(f2,1) < f1]
                    if d1_mode == "B":
                        nc.vector.tensor_scalar(
                            junkB[:, eslice(0, ee)],
                            Rr[(s, "g1")][:, eslice(0, ee)],
                            0.0, 0.0, Alu.is_gt, Alu.add,
                            accum_out=DM1[:, col:col + 1])
                    else:
                        nc.vector.scalar_tensor_tensor(
                            junkB[:, eslice(0, ee)], f2[:, eslice(0, ee)],
                            1.0, f1[:, eslice(0, ee)], Alu.max, Alu.is_lt,
                            accum_out=DM1[:, col:col + 1])

        # ---- software-pipelined emission ----
        WOVEN = set(CFG["weave"])

        def emit_loads(i):
            if i >= SPC:
                return
            if i == 0 and CFG["chunk_head"]:
                for half in (0, 1):
                    for c in range(C):
                        load_plane(0, c, half=half)
                return
            if i in WOVEN:
                return  # interleaved below
            for c in range(C):
                load_plane(i, c)

        def emit_woven_loads(i):
            for s in sorted(WOVEN):
                st = CFG["il_start"][s]
                if st <= i <= st + 1:
                    h = i - st
                    for c in range(C):
                        load_plane(s, c, half=h)

        def chunked(j, stage):
            if j == 0 and CFG["chunk_head"]:
                return [(0, 1), (1, 1), (2, 2)]
            if j == 1 and CFG["chunk_s1"] and stage == "f":
                return [(0, 2), (2, 2)]
            return (None,)

        def do_front(i):
            j = i - CFG["lag_front"]
            if 0 <= j < SPC and j not in WOVEN:
                stage_front(j, chunked(j, "f"))

        def do_mid(i):
            j = i - CFG["lag_mid"]
            if 0 <= j < SPC and j not in WOVEN:
                stage_mid(j, chunked(j, "m"))

        def do_back(i):
            j = i - CFG["lag_back"]
            if 0 <= j < SPC and j not in WOVEN:
                stage_back(j, chunked(j, "b"))

        # woven samples run half-granular front/mid, half-granular back
        WCH = [(0, 2), (2, 2)]

        def do_weave(i):
            for s in sorted(WOVEN):
                wl = CFG["weave_lag"][s]
                h = i - wl
                if 0 <= h < 2:
                    stage_front(s, (WCH[h],))
                h = i - wl - 1
                if 0 <= h < 2:
                    stage_mid(s, (WCH[h],))
                h = i - wl - 2
                if 0 <= h < 2:
                    stage_back(s, (WCH[h],))

        n_iter = max([SPC + CFG["lag_back"] + 1]
                     + [CFG["weave_lag"][s] + 2 + 3 for s in WOVEN])
        OMAP = {"b": do_back, "m": do_mid, "f": do_front}
        for i in range(n_iter):
            for ch in CFG["order"]:
                if ch == "l":
                    emit_loads(i)
                    emit_woven_loads(i)
                else:
                    OMAP[ch](i)
            do_weave(i)

        # ---- tail ----
        # column sums of the p1/r accumulators via gpsimd partition reduce
        O = cpool.tile([1, 40], f32, tag="O")
        S1 = cpool.tile([1, 64], f32, tag="S1")
        nc.gpsimd.tensor_reduce(S1[:, 0:32], RS1[:, :], C_AX, op=Alu.add)
        nc.gpsimd.tensor_reduce(S1[:, 32:64], RSr[:, :], C_AX, op=Alu.add)

        # DM2 = A - DM1
        teng = QENG[CFG.get("tail_tt", "dve")]
        DM2 = apool.tile([128, 32], f32, tag="DM2")
        teng.tensor_tensor(DM2[:, :], DMA_[:, :], DM1[:, :],
                           Alu.subtract)

        heights = []
        for li, DM in enumerate((DM1, DM2)):
            # pen = 1e6 where row absent; iota2[p, col] = (col%4)*128 + p
            pen = cpool.tile([128, 32], f32, tag=f"pen{li}")
            nc.vector.tensor_scalar(pen[:, :], DM[:, :], 0.5, 1e6,
                                    Alu.is_lt, Alu.mult)
            # ymin via max of negated iota: cols 32:64 of iota hold -h
            cminN = cpool.tile([128, 32], f32, tag=f"cminN{li}")
            teng.tensor_tensor(cminN[:, :], iota[:, 32:64], pen[:, :],
                               Alu.subtract)
            cmax = cpool.tile([128, 32], f32, tag=f"cmax{li}")
            teng.tensor_tensor(cmax[:, :], iota[:, 0:32], pen[:, :],
                               Alu.subtract)
            YminN = cpool.tile([1, 32], f32, tag=f"YminN{li}")
            Ymax = cpool.tile([1, 32], f32, tag=f"Ymax{li}")
            nc.gpsimd.tensor_reduce(YminN[:, :], cminN[:, :], C_AX,
                                    op=Alu.max)
            nc.gpsimd.tensor_reduce(Ymax[:, :], cmax[:, :], C_AX, op=Alu.max)
            yminN8 = cpool.tile([1, 8], f32, tag=f"yminN{li}")
            ymax8 = cpool.tile([1, 8], f32, tag=f"ymax{li}")
            nc.vector.tensor_reduce(
                yminN8[:, :],
                YminN[0:1, :].rearrange("p (s e) -> p s e", e=4),
                X_AX, op=Alu.max)
            nc.vector.tensor_reduce(
                ymax8[:, :],
                Ymax[0:1, :].rearrange("p (s e) -> p s e", e=4),
                X_AX, op=Alu.max)
            hL = cpool.tile([1, 8], f32, tag=f"h{li}")
            nc.vector.tensor_tensor(hL[:, :], ymax8[:, :], yminN8[:, :],
                                    Alu.add)
            nc.vector.tensor_scalar_max(hL[:, :], hL[:, :], 0.0)
            heights.append(hL)

        h_cup, h_disc = heights
        den = cpool.tile([1, 8], f32, tag="den")
        nc.vector.tensor_scalar_add(den[:, :], h_disc[:, :], 1e-6)
        rec = cpool.tile([1, 8], f32, tag="rec")
        nc.vector.reciprocal(rec[:, :], den[:, :])
        nc.vector.tensor_tensor(O[:, 0:8], h_cup[:, :], rec[:, :], Alu.mult)

        s1tot = cpool.tile([1, 8], f32, tag="s1tot")
        srtot = cpool.tile([1, 8], f32, tag="srtot")
        p2tot = cpool.tile([1, 8], f32, tag="p2tot")
        p2a = cpool.tile([1, 8], f32, tag="p2a")
        sc = 1.0 / HW

        nc.vector.tensor_reduce(
            s1tot[:, :],
            S1[0:1, 0:32].rearrange("p (s e) -> p s e", e=4),
            X_AX, op=Alu.add)
        nc.vector.tensor_reduce(
            srtot[:, :],
            S1[0:1, 32:64].rearrange("p (s e) -> p s e", e=4),
            X_AX, op=Alu.add)
        nc.vector.tensor_scalar(p2a[:, :], srtot[:, :], -1.0, HW,
                                Alu.mult, Alu.add)
        nc.vector.tensor_tensor(p2tot[:, :], p2a[:, :], s1tot[:, :],
                                Alu.subtract)
        nc.vector.tensor_scalar_mul(O[:, 8:16], p2tot[:, :], sc)
        nc.vector.tensor_scalar_mul(O[:, 16:24], s1tot[:, :], sc)
        nc.vector.tensor_scalar_mul(O[:, 24:32], p2tot[:, :], sc)
        nc.vector.tensor_scalar_mul(O[:, 32:40], s1tot[:, :], sc)

        nc.sync.dma_start(out[:, :], O[:, :])

    nc.finalize()
    return nc


def _get_nc():
    if "nc" not in _CACHE:
        _CACHE["nc"] = _build()
    return _CACHE["nc"]


def _host_inputs():
    # iota[p, s*4+e] = e*128 + p; cols 32:64 hold the negation
    iota = (np.arange(128, dtype=np.float32)[:, None]
            + 128.0 * np.tile(np.arange(4, dtype=np.float32), 8)[None, :])
    return (np.concatenate([iota, -iota], axis=1),)


def _run(seg_mask, trace=False):
    from concourse.bass_utils import run_bass_kernel_spmd

    x = np.ascontiguousarray(np.asarray(seg_mask, dtype=np.float32))
    assert x.shape == (B, C, H, W)
    (iota,) = _host_inputs()
    in_maps = [
        {"x": x[SPC * c:SPC * (c + 1)], "iota": iota}
        for c in range(NCORES)
    ]
    nc = _get_nc()
    res = run_bass_kernel_spmd(nc, in_maps, core_ids=list(range(NCORES)),
                               trace=trace)
    outs = []
    for c in range(NCORES):
        o = np.asarray(res.results[c]["out"]).reshape(5, SPC).T
        outs.append(o)
    full = np.concatenate(outs, axis=0).astype(np.float32)
    return full, res


def kernel(segmentation_mask):
    full, _ = _run(segmentation_mask, trace=False)
    return full
